# revision 53
# baseline (speedup 1.0000x reference)
"""Trainium2 Bass kernel for nn_AttractorState (decay-weighted outer-product state).

Reference computation (per batch b):
    C[b] = sum_t alpha^(S-1-t) * (W @ h_t + bias) outer e_t        (S = 8192)

Refactored to avoid materializing the projection and to keep the big
contraction over t in natural [t, d] layout:
    G[b]  = (w . H[b])^T @ PE[b]          # [d_model=512, d_model=512], w_t = alpha^(S-1-t)
    r[b]  = w^T @ PE[b]                   # [512]
    C[b]  = W @ G[b] + bias outer r[b]    # [d_state=512, d_model=512]

Sharding over 8 NeuronCores: (batch=4) x (d-half=2), fully collective-free.
Each core processes ALL 8192 tokens of one batch but only its 256 d-columns
of PE: G_half = (w . H)^T @ PE[:, dhalf] accumulates locally in PSUM, then
C_half = W @ G_half (+ b outer r_half), writing a (512, 256) slab.  The host
reassembles (4, 512, 512).  The t-contraction never crosses cores, so no
reduction, no collectives, no inter-core sync.

The bias path (b != 0) needs an extra rank-1 accumulation r = w^T @ PE and a
per-token M=1 matmul; setup_inputs() always produces b == 0, so the default
graph skips it entirely and a bias-capable graph is built only if a nonzero
b ever shows up.

Matmul operands are cast to bf16 on-chip (DVE/ACT, overlapped with DMA);
accumulation is fp32 in PSUM.
"""

import math
import sys

import numpy as np

for _p in ("/opt/trn_rl_repo", "/opt/trn_rl_repo/concourse"):
    if _p not in sys.path:
        sys.path.append(_p)

# Problem constants (hardcoded per harness contract).
B = 4
S = 8192
D = 512          # d_model
E = 512          # d_state
P = 128          # SBUF partitions
NCORES = 8
DH = D // 2      # 256, d-half owned per core
NT = S // P      # 64 t-tiles per core
# Chunk sizes (in 128-token t-tiles): small leading chunks so matmuls start
# early, tapering tail so the PE lag after the last DMA is tiny.  The last
# RES t-tiles are a "reserve" chunk: DMA'd + scaled early, matmuls deferred
# to the very end — a dense dependency-free burst that decouples the PE tail
# from the last DMA arrival.
RES = 6
SIZES = [1, 2, 3, 4, 6, 6, 6, 6, 6, 6, 6, 6]
assert sum(SIZES) == NT - RES
CH = len(SIZES)
TPC = max(SIZES)  # buffer sizing

_GRAPH_CACHE = {}


def _decay_weights():
    # Match reference: alpha = f32(exp(-pi/S)); w = exp((S-1-t) * log(alpha)) in f32.
    alpha = np.float32(math.exp(-math.pi / S))
    t = np.arange(S, dtype=np.float32)
    w = np.exp((np.float32(S - 1.0) - t) * np.log(alpha)).astype(np.float32)
    return w


def _build(bias: bool):
    key = ("bias" if bias else "nobias")
    if key in _GRAPH_CACHE:
        return _GRAPH_CACHE[key]

    import concourse.bass as bass  # noqa: F401
    import concourse.mybir as mybir
    import concourse.tile as tile
    from concourse import bacc

    f32 = mybir.dt.float32
    bf16 = mybir.dt.bfloat16
    AF = mybir.ActivationFunctionType

    nc = bacc.Bacc("TRN2", target_bir_lowering=False)

    h_ext = nc.declare_dram_parameter("h", [S, D], f32, isOutput=False)
    pe_ext = nc.declare_dram_parameter("pe", [S, DH], f32, isOutput=False)
    w_ext = nc.declare_dram_parameter("W", [E, D], f32, isOutput=False)
    b_ext = nc.declare_dram_parameter("b", [E], f32, isOutput=False)
    wdec_ext = nc.declare_dram_parameter("wdec", [P, NT], f32, isOutput=False)
    out_ext = nc.declare_dram_parameter("out", [E, DH], f32, isOutput=True)

    import ml_dtypes

    ident = nc.inline_tensor(np.eye(P, dtype=ml_dtypes.bfloat16), "ident")

    h_re = h_ext.ap().rearrange("(n p) d -> p n d", p=P)
    pe_re = pe_ext.ap().rearrange("(n p) d -> p n d", p=P)
    w_re = w_ext.ap().rearrange("(c p) e -> p c e", p=P)

    with tile.TileContext(nc) as tc:
        with (
            tc.tile_pool(name="consts", bufs=1) as consts,
            tc.tile_pool(name="io", bufs=4) as io,
            tc.tile_pool(name="acc", bufs=1, space="PSUM") as acc_pool,
            tc.tile_pool(name="pst", bufs=2, space="PSUM") as pst,
        ):
            # ---- first input chunks go out before anything else ----
            # (wdec rides the otherwise-idle gpsimd SWDGE so h0 leads the sync ring)
            wdec_sb = consts.tile([P, NT], f32)
            nc.gpsimd.dma_start(wdec_sb[:], wdec_ext[:, :])
            starts = [sum(SIZES[:j]) for j in range(CH)]
            h_tiles = []
            pe_tiles = []
            PRE = 4
            for j in range(PRE):
                n0, w = starts[j], SIZES[j]
                h_t = io.tile([P, TPC, D], f32, tag="h", name=f"h_t{j}")
                pe_t = io.tile([P, TPC, DH], f32, tag="pe", name=f"pe_t{j}")
                nc.sync.dma_start(h_t[:, 0:w, :], h_re[:, n0:n0 + w, :])
                nc.scalar.dma_start(pe_t[:, 0:w, :], pe_re[:, n0:n0 + w, :])
                h_tiles.append(h_t)
                pe_tiles.append(pe_t)

            # ---- reserve chunk: tiles [NT-RES, NT), fetched + prepped early ----
            R0 = NT - RES
            h_r = io.tile([P, RES, D], f32, tag="hr", bufs=1)
            pe_r = io.tile([P, RES, DH], f32, tag="per", bufs=1)
            nc.sync.dma_start(h_r[:], h_re[:, R0:NT, :])
            nc.scalar.dma_start(pe_r[:], pe_re[:, R0:NT, :])
            hw_r = io.tile([P, RES, D], bf16, tag="hwr", bufs=1)
            pew_r = io.tile([P, RES, DH], bf16, tag="pewr", bufs=1)
            nc.vector.tensor_tensor(
                out=hw_r[:],
                in0=h_r[:],
                in1=wdec_sb[:, R0:NT].unsqueeze(-1).to_broadcast((P, RES, D)),
                op=mybir.AluOpType.mult,
            )
            nc.scalar.activation(pew_r[:], pe_r[:], AF.Copy)

            # ---- constants ----
            ident_sb = consts.tile([P, P], bf16)
            nc.sync.dma_start(ident_sb[:], ident[:, :])
            w_sb = consts.tile([P, 4, D], f32)      # w_sb[p, c, e] = W[c*128+p, e]
            nc.sync.dma_start(w_sb[:], w_re)
            w_bf = consts.tile([P, 4, D], bf16)
            nc.vector.tensor_copy(w_bf[:], w_sb[:])
            if bias:
                wdec_bf = consts.tile([P, NT], bf16)
                nc.vector.tensor_copy(wdec_bf[:], wdec_sb[:])
                b_sb = consts.tile([1, E], f32)
                nc.sync.dma_start(b_sb[:], b_ext.ap().unsqueeze(0))
                b_bf = consts.tile([1, E], bf16)
                nc.vector.tensor_copy(b_bf[:], b_sb[:])

            # ---- G (/ r) accumulation over all 8192 tokens ----
            g_ps = [
                acc_pool.tile([P, DH], f32, tag=f"g{k}", name=f"g_ps{k}")
                for k in range(4)
            ]
            if bias:
                r_ps = acc_pool.tile([1, DH], f32, tag="r")

            for j in range(CH):
                n0, w = starts[j], SIZES[j]
                if j < PRE:
                    h_t, pe_t = h_tiles[j], pe_tiles[j]
                else:
                    h_t = io.tile([P, TPC, D], f32, tag="h")
                    pe_t = io.tile([P, TPC, DH], f32, tag="pe")
                    nc.sync.dma_start(h_t[:, 0:w, :], h_re[:, n0:n0 + w, :])
                    nc.scalar.dma_start(pe_t[:, 0:w, :], pe_re[:, n0:n0 + w, :])
                hw_t = io.tile([P, TPC, D], bf16, tag="hw")
                pew_t = io.tile([P, TPC, DH], bf16, tag="pew")
                # whole-chunk decay-scale of h on DVE (wdec broadcast along d)
                # and whole-chunk cast of pe on ACT
                nc.vector.tensor_tensor(
                    out=hw_t[:, 0:w, :],
                    in0=h_t[:, 0:w, :],
                    in1=wdec_sb[:, n0:n0 + w].unsqueeze(-1).to_broadcast((P, w, D)),
                    op=mybir.AluOpType.mult,
                )
                nc.scalar.activation(pew_t[:, 0:w, :], pe_t[:, 0:w, :], AF.Copy)
                for i in range(w):
                    n = n0 + i
                    first = n == 0
                    for k in range(4):
                        nc.tensor.matmul(
                            g_ps[k][:],
                            hw_t[:, i, k * P:(k + 1) * P],
                            pew_t[:, i, :],
                            start=first,
                            stop=False,
                        )
                    if bias:
                        nc.tensor.matmul(
                            r_ps[:],
                            wdec_bf[:, n:n + 1],
                            pew_t[:, i, :],
                            start=first,
                            stop=False,
                        )
                if j == 0:
                    # W^T transposes (bf16), scheduled after the first chunk's matmuls
                    wt_sb = consts.tile([P, 4, E], bf16)  # wt_sb[p,c,s] = W[s, c*128+p]
                    for ce in range(4):
                        for cs in range(4):
                            tp = pst.tile([P, P], bf16, tag="tp")
                            nc.tensor.transpose(
                                tp[:], w_bf[:, cs, ce * P:(ce + 1) * P], ident_sb[:]
                            )
                            nc.vector.tensor_copy(
                                wt_sb[:, ce, cs * P:(cs + 1) * P], tp[:]
                            )

            # ---- reserve chunk matmuls: dense, all inputs long resident ----
            for i in range(RES):
                n = R0 + i
                last = n == NT - 1
                for k in range(4):
                    nc.tensor.matmul(
                        g_ps[k][:],
                        hw_r[:, i, k * P:(k + 1) * P],
                        pew_r[:, i, :],
                        start=False,
                        stop=last,
                    )
                if bias:
                    nc.tensor.matmul(
                        r_ps[:],
                        wdec_bf[:, n:n + 1],
                        pew_r[:, i, :],
                        start=False,
                        stop=last,
                    )

            # ---- G -> SBUF (bf16) ----
            gred_bf = consts.tile([P, 4, DH], bf16)
            for k in range(4):
                nc.vector.tensor_copy(gred_bf[:, k, :], g_ps[k][:])
            if bias:
                rred_bf = consts.tile([1, DH], bf16)
                nc.vector.tensor_copy(rred_bf[:], r_ps[:])

            # ---- finish C_half = W @ G_half (+ b outer r_half) ----
            for cs in range(4):
                c_ps = pst.tile([P, DH], f32, tag="cps", bufs=1)
                for ce in range(4):
                    nc.tensor.matmul(
                        c_ps[:],
                        wt_sb[:, ce, cs * P:(cs + 1) * P],
                        gred_bf[:, ce, :],
                        start=(ce == 0),
                        stop=(not bias and ce == 3),
                    )
                if bias:
                    nc.tensor.matmul(
                        c_ps[:],
                        b_bf[0:1, cs * P:(cs + 1) * P],
                        rred_bf[:],
                        start=False,
                        stop=True,
                    )
                c_sb = io.tile([P, DH], f32, tag="csb")
                nc.vector.tensor_copy(c_sb[:], c_ps[:])
                nc.sync.dma_start(out_ext[cs * P:(cs + 1) * P, :], c_sb[:])

    nc.compile()
    _GRAPH_CACHE[key] = nc
    return nc


def _in_maps(hidden_states, positional_encodings, W, b):
    w_full = _decay_weights()
    wdec = np.ascontiguousarray(w_full.reshape(NT, P).T, dtype=np.float32)
    W_c = np.ascontiguousarray(W, dtype=np.float32)
    b_c = np.ascontiguousarray(b, dtype=np.float32)
    maps = []
    for c in range(NCORES):
        bi, dh = c // 2, c % 2
        maps.append(
            {
                "h": np.ascontiguousarray(hidden_states[bi], dtype=np.float32),
                "pe": np.ascontiguousarray(
                    positional_encodings[bi, :, dh * DH:(dh + 1) * DH],
                    dtype=np.float32,
                ),
                "W": W_c,
                "b": b_c,
                "wdec": wdec,
            }
        )
    return maps


def _assemble(results):
    out = np.empty((B, E, D), dtype=np.float32)
    for c in range(NCORES):
        bi, dh = c // 2, c % 2
        out[bi, :, dh * DH:(dh + 1) * DH] = results[c]["out"]
    return out


def run(hidden_states, positional_encodings, W, b, trace=False, **trace_kwargs):
    from concourse.bass_utils import run_bass_kernel_spmd

    nc = _build(bias=bool(np.any(np.asarray(b) != 0)))
    maps = _in_maps(hidden_states, positional_encodings, W, b)
    res = run_bass_kernel_spmd(
        nc, maps, core_ids=list(range(NCORES)), trace=trace, **trace_kwargs
    )
    return _assemble(res.results), res


def kernel(hidden_states, positional_encodings, W, b):
    out, _ = run(hidden_states, positional_encodings, W, b, trace=False)
    return out


# revision 56
# speedup vs baseline: 1.0517x; 1.0517x over previous
"""Trainium2 Bass kernel for nn_AttractorState (decay-weighted outer-product state).

Reference computation (per batch b):
    C[b] = sum_t alpha^(S-1-t) * (W @ h_t + bias) outer e_t        (S = 8192)

Refactored to avoid materializing the projection and to keep the big
contraction over t in natural [t, d] layout:
    G[b]  = (w . H[b])^T @ PE[b]          # [d_model=512, d_model=512], w_t = alpha^(S-1-t)
    r[b]  = w^T @ PE[b]                   # [512]
    C[b]  = W @ G[b] + bias outer r[b]    # [d_state=512, d_model=512]

Sharding over 8 NeuronCores: (batch=4) x (d-half=2), fully collective-free.
Each core processes ALL 8192 tokens of one batch but only its 256 d-columns
of PE: G_half = (w . H)^T @ PE[:, dhalf] accumulates locally in PSUM, then
C_half = W @ G_half (+ b outer r_half), writing a (512, 256) slab.  The host
reassembles (4, 512, 512).  The t-contraction never crosses cores, so no
reduction, no collectives, no inter-core sync.

The bias path (b != 0) needs an extra rank-1 accumulation r = w^T @ PE and a
per-token M=1 matmul; setup_inputs() always produces b == 0, so the default
graph skips it entirely and a bias-capable graph is built only if a nonzero
b ever shows up.

Matmul operands are cast to bf16 on-chip (DVE/ACT, overlapped with DMA);
accumulation is fp32 in PSUM.
"""

import math
import sys

import numpy as np

for _p in ("/opt/trn_rl_repo", "/opt/trn_rl_repo/concourse"):
    if _p not in sys.path:
        sys.path.append(_p)

# Problem constants (hardcoded per harness contract).
B = 4
S = 8192
D = 512          # d_model
E = 512          # d_state
P = 128          # SBUF partitions
NCORES = 8
DH = D // 2      # 256, d-half owned per core
NT = S // P      # 64 t-tiles per core
# Chunk sizes (in 128-token t-tiles): small leading chunks so matmuls start
# early, tapering tail so the PE lag after the last DMA is tiny.  The last
# RES t-tiles are a "reserve" chunk: DMA'd + scaled early, matmuls deferred
# to the very end — a dense dependency-free burst that decouples the PE tail
# from the last DMA arrival.
RES = 6
SIZES = [1, 2, 3, 4, 6, 6, 6, 6, 6, 6, 6, 4, 2]
assert sum(SIZES) == NT - RES
CH = len(SIZES)
TPC = max(SIZES)  # buffer sizing

_GRAPH_CACHE = {}


def _decay_weights():
    # Match reference: alpha = f32(exp(-pi/S)); w = exp((S-1-t) * log(alpha)) in f32.
    alpha = np.float32(math.exp(-math.pi / S))
    t = np.arange(S, dtype=np.float32)
    w = np.exp((np.float32(S - 1.0) - t) * np.log(alpha)).astype(np.float32)
    return w


def _build(bias: bool):
    key = ("bias" if bias else "nobias")
    if key in _GRAPH_CACHE:
        return _GRAPH_CACHE[key]

    import concourse.bass as bass  # noqa: F401
    import concourse.mybir as mybir
    import concourse.tile as tile
    from concourse import bacc

    f32 = mybir.dt.float32
    bf16 = mybir.dt.bfloat16
    AF = mybir.ActivationFunctionType

    nc = bacc.Bacc("TRN2", target_bir_lowering=False)

    h_ext = nc.declare_dram_parameter("h", [S, D], f32, isOutput=False)
    pe_ext = nc.declare_dram_parameter("pe", [S, DH], f32, isOutput=False)
    w_ext = nc.declare_dram_parameter("W", [E, D], f32, isOutput=False)
    b_ext = nc.declare_dram_parameter("b", [E], f32, isOutput=False)
    wdec_ext = nc.declare_dram_parameter("wdec", [P, NT], f32, isOutput=False)
    out_ext = nc.declare_dram_parameter("out", [E, DH], f32, isOutput=True)

    import ml_dtypes

    ident = nc.inline_tensor(np.eye(P, dtype=ml_dtypes.bfloat16), "ident")

    h_re = h_ext.ap().rearrange("(n p) d -> p n d", p=P)
    pe_re = pe_ext.ap().rearrange("(n p) d -> p n d", p=P)
    w_re = w_ext.ap().rearrange("(c p) e -> p c e", p=P)

    with tile.TileContext(nc) as tc:
        with (
            tc.tile_pool(name="consts", bufs=1) as consts,
            tc.tile_pool(name="io", bufs=4) as io,
            tc.tile_pool(name="acc", bufs=1, space="PSUM") as acc_pool,
            tc.tile_pool(name="pst", bufs=2, space="PSUM") as pst,
        ):
            # ---- first input chunks go out before anything else ----
            # (wdec rides the otherwise-idle gpsimd SWDGE so h0 leads the sync ring)
            wdec_sb = consts.tile([P, NT], f32)
            nc.gpsimd.dma_start(wdec_sb[:], wdec_ext[:, :])
            starts = [sum(SIZES[:j]) for j in range(CH)]
            h_tiles = []
            pe_tiles = []
            PRE = 4
            for j in range(PRE):
                n0, w = starts[j], SIZES[j]
                h_t = io.tile([P, TPC, D], f32, tag="h", name=f"h_t{j}")
                pe_t = io.tile([P, TPC, DH], f32, tag="pe", name=f"pe_t{j}")
                nc.sync.dma_start(h_t[:, 0:w, :], h_re[:, n0:n0 + w, :])
                nc.scalar.dma_start(pe_t[:, 0:w, :], pe_re[:, n0:n0 + w, :])
                h_tiles.append(h_t)
                pe_tiles.append(pe_t)

            # ---- reserve chunk: tiles [NT-RES, NT), fetched + prepped early ----
            R0 = NT - RES
            h_r = io.tile([P, RES, D], f32, tag="hr", bufs=1)
            pe_r = io.tile([P, RES, DH], f32, tag="per", bufs=1)
            nc.sync.dma_start(h_r[:], h_re[:, R0:NT, :])
            nc.scalar.dma_start(pe_r[:], pe_re[:, R0:NT, :])
            hw_r = io.tile([P, RES, D], bf16, tag="hwr", bufs=1)
            pew_r = io.tile([P, RES, DH], bf16, tag="pewr", bufs=1)
            nc.vector.tensor_tensor(
                out=hw_r[:],
                in0=h_r[:],
                in1=wdec_sb[:, R0:NT].unsqueeze(-1).to_broadcast((P, RES, D)),
                op=mybir.AluOpType.mult,
            )
            nc.scalar.activation(pew_r[:], pe_r[:], AF.Copy)

            # ---- constants ----
            ident_sb = consts.tile([P, P], bf16)
            nc.sync.dma_start(ident_sb[:], ident[:, :])
            w_sb = consts.tile([P, 4, D], f32)      # w_sb[p, c, e] = W[c*128+p, e]
            nc.sync.dma_start(w_sb[:], w_re)
            w_bf = consts.tile([P, 4, D], bf16)
            nc.vector.tensor_copy(w_bf[:], w_sb[:])
            if bias:
                wdec_bf = consts.tile([P, NT], bf16)
                nc.vector.tensor_copy(wdec_bf[:], wdec_sb[:])
                b_sb = consts.tile([1, E], f32)
                nc.sync.dma_start(b_sb[:], b_ext.ap().unsqueeze(0))
                b_bf = consts.tile([1, E], bf16)
                nc.vector.tensor_copy(b_bf[:], b_sb[:])

            # ---- G (/ r) accumulation over all 8192 tokens ----
            g_ps = [
                acc_pool.tile([P, DH], f32, tag=f"g{k}", name=f"g_ps{k}")
                for k in range(4)
            ]
            if bias:
                r_ps = acc_pool.tile([1, DH], f32, tag="r")

            for j in range(CH):
                n0, w = starts[j], SIZES[j]
                if j < PRE:
                    h_t, pe_t = h_tiles[j], pe_tiles[j]
                else:
                    h_t = io.tile([P, TPC, D], f32, tag="h")
                    pe_t = io.tile([P, TPC, DH], f32, tag="pe")
                    nc.sync.dma_start(h_t[:, 0:w, :], h_re[:, n0:n0 + w, :])
                    nc.scalar.dma_start(pe_t[:, 0:w, :], pe_re[:, n0:n0 + w, :])
                hw_t = io.tile([P, TPC, D], bf16, tag="hw")
                pew_t = io.tile([P, TPC, DH], bf16, tag="pew")
                # decay-scale of h on DVE (wdec broadcast along d) and cast of
                # pe on ACT, in half-chunk pieces so the chunk's first matmuls
                # start after half the scale latency
                halves = [(0, (w + 1) // 2), ((w + 1) // 2, w)] if w > 2 else [(0, w)]
                for (a, bnd) in halves:
                    if bnd <= a:
                        continue
                    wd = bnd - a
                    nc.vector.tensor_tensor(
                        out=hw_t[:, a:bnd, :],
                        in0=h_t[:, a:bnd, :],
                        in1=wdec_sb[:, n0 + a:n0 + bnd].unsqueeze(-1).to_broadcast(
                            (P, wd, D)
                        ),
                        op=mybir.AluOpType.mult,
                    )
                    nc.scalar.activation(pew_t[:, a:bnd, :], pe_t[:, a:bnd, :], AF.Copy)
                for i in range(w):
                    n = n0 + i
                    first = n == 0
                    for k in range(4):
                        nc.tensor.matmul(
                            g_ps[k][:],
                            hw_t[:, i, k * P:(k + 1) * P],
                            pew_t[:, i, :],
                            start=first,
                            stop=False,
                        )
                    if bias:
                        nc.tensor.matmul(
                            r_ps[:],
                            wdec_bf[:, n:n + 1],
                            pew_t[:, i, :],
                            start=first,
                            stop=False,
                        )
                if j == 0:
                    # W^T transposes (bf16), scheduled after the first chunk's matmuls
                    wt_sb = consts.tile([P, 4, E], bf16)  # wt_sb[p,c,s] = W[s, c*128+p]
                    for ce in range(4):
                        for cs in range(4):
                            tp = pst.tile([P, P], bf16, tag="tp")
                            nc.tensor.transpose(
                                tp[:], w_bf[:, cs, ce * P:(ce + 1) * P], ident_sb[:]
                            )
                            nc.vector.tensor_copy(
                                wt_sb[:, ce, cs * P:(cs + 1) * P], tp[:]
                            )

            # ---- reserve chunk matmuls: dense, all inputs long resident ----
            for i in range(RES):
                n = R0 + i
                last = n == NT - 1
                for k in range(4):
                    nc.tensor.matmul(
                        g_ps[k][:],
                        hw_r[:, i, k * P:(k + 1) * P],
                        pew_r[:, i, :],
                        start=False,
                        stop=last,
                    )
                if bias:
                    nc.tensor.matmul(
                        r_ps[:],
                        wdec_bf[:, n:n + 1],
                        pew_r[:, i, :],
                        start=False,
                        stop=last,
                    )

            # ---- G -> SBUF (bf16) ----
            gred_bf = consts.tile([P, 4, DH], bf16)
            for k in range(4):
                nc.vector.tensor_copy(gred_bf[:, k, :], g_ps[k][:])
            if bias:
                rred_bf = consts.tile([1, DH], bf16)
                nc.vector.tensor_copy(rred_bf[:], r_ps[:])

            # ---- finish C_half = W @ G_half (+ b outer r_half) ----
            for cs in range(4):
                c_ps = pst.tile([P, DH], f32, tag="cps", bufs=1)
                for ce in range(4):
                    nc.tensor.matmul(
                        c_ps[:],
                        wt_sb[:, ce, cs * P:(cs + 1) * P],
                        gred_bf[:, ce, :],
                        start=(ce == 0),
                        stop=(not bias and ce == 3),
                    )
                if bias:
                    nc.tensor.matmul(
                        c_ps[:],
                        b_bf[0:1, cs * P:(cs + 1) * P],
                        rred_bf[:],
                        start=False,
                        stop=True,
                    )
                c_sb = io.tile([P, DH], f32, tag="csb")
                nc.vector.tensor_copy(c_sb[:], c_ps[:])
                eng = nc.sync if cs % 2 == 0 else nc.scalar
                eng.dma_start(out_ext[cs * P:(cs + 1) * P, :], c_sb[:])

    nc.compile()
    _GRAPH_CACHE[key] = nc
    return nc


def _in_maps(hidden_states, positional_encodings, W, b):
    w_full = _decay_weights()
    wdec = np.ascontiguousarray(w_full.reshape(NT, P).T, dtype=np.float32)
    W_c = np.ascontiguousarray(W, dtype=np.float32)
    b_c = np.ascontiguousarray(b, dtype=np.float32)
    maps = []
    for c in range(NCORES):
        bi, dh = c // 2, c % 2
        maps.append(
            {
                "h": np.ascontiguousarray(hidden_states[bi], dtype=np.float32),
                "pe": np.ascontiguousarray(
                    positional_encodings[bi, :, dh * DH:(dh + 1) * DH],
                    dtype=np.float32,
                ),
                "W": W_c,
                "b": b_c,
                "wdec": wdec,
            }
        )
    return maps


def _assemble(results):
    out = np.empty((B, E, D), dtype=np.float32)
    for c in range(NCORES):
        bi, dh = c // 2, c % 2
        out[bi, :, dh * DH:(dh + 1) * DH] = results[c]["out"]
    return out


def run(hidden_states, positional_encodings, W, b, trace=False, **trace_kwargs):
    from concourse.bass_utils import run_bass_kernel_spmd

    nc = _build(bias=bool(np.any(np.asarray(b) != 0)))
    maps = _in_maps(hidden_states, positional_encodings, W, b)
    res = run_bass_kernel_spmd(
        nc, maps, core_ids=list(range(NCORES)), trace=trace, **trace_kwargs
    )
    return _assemble(res.results), res


def kernel(hidden_states, positional_encodings, W, b):
    out, _ = run(hidden_states, positional_encodings, W, b, trace=False)
    return out


# revision 59
# speedup vs baseline: 1.0554x; 1.0036x over previous
"""Trainium2 Bass kernel for nn_AttractorState (decay-weighted outer-product state).

Reference computation (per batch b):
    C[b] = sum_t alpha^(S-1-t) * (W @ h_t + bias) outer e_t        (S = 8192)

Refactored to avoid materializing the projection and to keep the big
contraction over t in natural [t, d] layout:
    G[b]  = (w . H[b])^T @ PE[b]          # [d_model=512, d_model=512], w_t = alpha^(S-1-t)
    r[b]  = w^T @ PE[b]                   # [512]
    C[b]  = W @ G[b] + bias outer r[b]    # [d_state=512, d_model=512]

Sharding over 8 NeuronCores: (batch=4) x (d-half=2), fully collective-free.
Each core processes ALL 8192 tokens of one batch but only its 256 d-columns
of PE: G_half = (w . H)^T @ PE[:, dhalf] accumulates locally in PSUM, then
C_half = W @ G_half (+ b outer r_half), writing a (512, 256) slab.  The host
reassembles (4, 512, 512).  The t-contraction never crosses cores, so no
reduction, no collectives, no inter-core sync.

The bias path (b != 0) needs an extra rank-1 accumulation r = w^T @ PE and a
per-token M=1 matmul; setup_inputs() always produces b == 0, so the default
graph skips it entirely and a bias-capable graph is built only if a nonzero
b ever shows up.

Matmul operands are cast to bf16 on-chip (DVE/ACT, overlapped with DMA);
accumulation is fp32 in PSUM.
"""

import math
import sys

import numpy as np

for _p in ("/opt/trn_rl_repo", "/opt/trn_rl_repo/concourse"):
    if _p not in sys.path:
        sys.path.append(_p)

# Problem constants (hardcoded per harness contract).
B = 4
S = 8192
D = 512          # d_model
E = 512          # d_state
P = 128          # SBUF partitions
NCORES = 8
DH = D // 2      # 256, d-half owned per core
NT = S // P      # 64 t-tiles per core
# Chunk sizes (in 128-token t-tiles): small leading chunks so matmuls start
# early, tapering tail so the PE lag after the last DMA is tiny.  The last
# RES t-tiles are a "reserve" chunk: DMA'd + scaled early, matmuls deferred
# to the very end — a dense dependency-free burst that decouples the PE tail
# from the last DMA arrival.
RES = 6
SIZES = [1, 2, 3, 4, 8, 4, 8, 4, 8, 4, 6, 4, 2]
assert sum(SIZES) == NT - RES
CH = len(SIZES)
TPC = max(SIZES)  # buffer sizing

_GRAPH_CACHE = {}


def _decay_weights():
    # Match reference: alpha = f32(exp(-pi/S)); w = exp((S-1-t) * log(alpha)) in f32.
    alpha = np.float32(math.exp(-math.pi / S))
    t = np.arange(S, dtype=np.float32)
    w = np.exp((np.float32(S - 1.0) - t) * np.log(alpha)).astype(np.float32)
    return w


def _build(bias: bool):
    key = ("bias" if bias else "nobias")
    if key in _GRAPH_CACHE:
        return _GRAPH_CACHE[key]

    import concourse.bass as bass  # noqa: F401
    import concourse.mybir as mybir
    import concourse.tile as tile
    from concourse import bacc

    f32 = mybir.dt.float32
    bf16 = mybir.dt.bfloat16
    AF = mybir.ActivationFunctionType

    nc = bacc.Bacc("TRN2", target_bir_lowering=False)

    h_ext = nc.declare_dram_parameter("h", [S, D], f32, isOutput=False)
    pe_ext = nc.declare_dram_parameter("pe", [S, DH], f32, isOutput=False)
    w_ext = nc.declare_dram_parameter("W", [E, D], f32, isOutput=False)
    b_ext = nc.declare_dram_parameter("b", [E], f32, isOutput=False)
    wdec_ext = nc.declare_dram_parameter("wdec", [P, NT], f32, isOutput=False)
    out_ext = nc.declare_dram_parameter("out", [E, DH], f32, isOutput=True)

    import ml_dtypes

    ident = nc.inline_tensor(np.eye(P, dtype=ml_dtypes.bfloat16), "ident")

    h_re = h_ext.ap().rearrange("(n p) d -> p n d", p=P)
    pe_re = pe_ext.ap().rearrange("(n p) d -> p n d", p=P)
    w_re = w_ext.ap().rearrange("(c p) e -> p c e", p=P)

    with tile.TileContext(nc) as tc:
        with (
            tc.tile_pool(name="consts", bufs=1) as consts,
            tc.tile_pool(name="io", bufs=4) as io,
            tc.tile_pool(name="acc", bufs=1, space="PSUM") as acc_pool,
            tc.tile_pool(name="pst", bufs=2, space="PSUM") as pst,
        ):
            # ---- first input chunks go out before anything else ----
            # (wdec rides the otherwise-idle gpsimd SWDGE so h0 leads the sync ring)
            wdec_sb = consts.tile([P, NT], f32)
            nc.gpsimd.dma_start(wdec_sb[:], wdec_ext[:, :])
            starts = [sum(SIZES[:j]) for j in range(CH)]
            h_tiles = []
            pe_tiles = []
            PRE = 4
            for j in range(PRE):
                n0, w = starts[j], SIZES[j]
                h_t = io.tile([P, TPC, D], f32, tag="h", name=f"h_t{j}")
                pe_t = io.tile([P, TPC, DH], f32, tag="pe", name=f"pe_t{j}")
                nc.sync.dma_start(h_t[:, 0:w, :], h_re[:, n0:n0 + w, :])
                nc.scalar.dma_start(pe_t[:, 0:w, :], pe_re[:, n0:n0 + w, :])
                h_tiles.append(h_t)
                pe_tiles.append(pe_t)

            # ---- reserve chunk: tiles [NT-RES, NT), fetched + prepped early ----
            R0 = NT - RES
            h_r = io.tile([P, RES, D], f32, tag="hr", bufs=1)
            pe_r = io.tile([P, RES, DH], f32, tag="per", bufs=1)
            nc.sync.dma_start(h_r[:], h_re[:, R0:NT, :])
            nc.scalar.dma_start(pe_r[:], pe_re[:, R0:NT, :])
            hw_r = io.tile([P, RES, D], bf16, tag="hwr", bufs=1)
            pew_r = io.tile([P, RES, DH], bf16, tag="pewr", bufs=1)
            nc.vector.tensor_tensor(
                out=hw_r[:],
                in0=h_r[:],
                in1=wdec_sb[:, R0:NT].unsqueeze(-1).to_broadcast((P, RES, D)),
                op=mybir.AluOpType.mult,
            )
            nc.scalar.activation(pew_r[:], pe_r[:], AF.Copy)

            # ---- constants ----
            ident_sb = consts.tile([P, P], bf16)
            nc.sync.dma_start(ident_sb[:], ident[:, :])
            w_sb = consts.tile([P, 4, D], f32)      # w_sb[p, c, e] = W[c*128+p, e]
            nc.sync.dma_start(w_sb[:], w_re)
            w_bf = consts.tile([P, 4, D], bf16)
            nc.vector.tensor_copy(w_bf[:], w_sb[:])
            if bias:
                wdec_bf = consts.tile([P, NT], bf16)
                nc.vector.tensor_copy(wdec_bf[:], wdec_sb[:])
                b_sb = consts.tile([1, E], f32)
                nc.sync.dma_start(b_sb[:], b_ext.ap().unsqueeze(0))
                b_bf = consts.tile([1, E], bf16)
                nc.vector.tensor_copy(b_bf[:], b_sb[:])

            # ---- G (/ r) accumulation over all 8192 tokens ----
            g_ps = [
                acc_pool.tile([P, DH], f32, tag=f"g{k}", name=f"g_ps{k}")
                for k in range(4)
            ]
            if bias:
                r_ps = acc_pool.tile([1, DH], f32, tag="r")

            for j in range(CH):
                n0, w = starts[j], SIZES[j]
                if j < PRE:
                    h_t, pe_t = h_tiles[j], pe_tiles[j]
                else:
                    h_t = io.tile([P, TPC, D], f32, tag="h")
                    pe_t = io.tile([P, TPC, DH], f32, tag="pe")
                    nc.sync.dma_start(h_t[:, 0:w, :], h_re[:, n0:n0 + w, :])
                    nc.scalar.dma_start(pe_t[:, 0:w, :], pe_re[:, n0:n0 + w, :])
                hw_t = io.tile([P, TPC, D], bf16, tag="hw")
                pew_t = io.tile([P, TPC, DH], bf16, tag="pew")
                # whole-chunk decay-scale of h on DVE (wdec broadcast along d)
                # and whole-chunk cast of pe on ACT
                nc.vector.tensor_tensor(
                    out=hw_t[:, 0:w, :],
                    in0=h_t[:, 0:w, :],
                    in1=wdec_sb[:, n0:n0 + w].unsqueeze(-1).to_broadcast((P, w, D)),
                    op=mybir.AluOpType.mult,
                )
                nc.scalar.activation(pew_t[:, 0:w, :], pe_t[:, 0:w, :], AF.Copy)
                for i in range(w):
                    n = n0 + i
                    first = n == 0
                    for k in range(4):
                        nc.tensor.matmul(
                            g_ps[k][:],
                            hw_t[:, i, k * P:(k + 1) * P],
                            pew_t[:, i, :],
                            start=first,
                            stop=False,
                        )
                    if bias:
                        nc.tensor.matmul(
                            r_ps[:],
                            wdec_bf[:, n:n + 1],
                            pew_t[:, i, :],
                            start=first,
                            stop=False,
                        )
                if j == 0:
                    # W^T transposes (bf16), scheduled after the first chunk's matmuls
                    wt_sb = consts.tile([P, 4, E], bf16)  # wt_sb[p,c,s] = W[s, c*128+p]
                    for ce in range(4):
                        for cs in range(4):
                            tp = pst.tile([P, P], bf16, tag="tp")
                            nc.tensor.transpose(
                                tp[:], w_bf[:, cs, ce * P:(ce + 1) * P], ident_sb[:]
                            )
                            nc.vector.tensor_copy(
                                wt_sb[:, ce, cs * P:(cs + 1) * P], tp[:]
                            )

            # ---- reserve chunk matmuls: dense, all inputs long resident ----
            for i in range(RES):
                n = R0 + i
                last = n == NT - 1
                for k in range(4):
                    nc.tensor.matmul(
                        g_ps[k][:],
                        hw_r[:, i, k * P:(k + 1) * P],
                        pew_r[:, i, :],
                        start=False,
                        stop=last,
                    )
                if bias:
                    nc.tensor.matmul(
                        r_ps[:],
                        wdec_bf[:, n:n + 1],
                        pew_r[:, i, :],
                        start=False,
                        stop=last,
                    )

            # ---- G -> SBUF (bf16) ----
            gred_bf = consts.tile([P, 4, DH], bf16)
            for k in range(4):
                nc.vector.tensor_copy(gred_bf[:, k, :], g_ps[k][:])
            if bias:
                rred_bf = consts.tile([1, DH], bf16)
                nc.vector.tensor_copy(rred_bf[:], r_ps[:])

            # ---- finish C_half = W @ G_half (+ b outer r_half) ----
            for cs in range(4):
                c_ps = pst.tile([P, DH], f32, tag="cps", bufs=(1 if bias else 2))
                for ce in range(4):
                    nc.tensor.matmul(
                        c_ps[:],
                        wt_sb[:, ce, cs * P:(cs + 1) * P],
                        gred_bf[:, ce, :],
                        start=(ce == 0),
                        stop=(not bias and ce == 3),
                    )
                if bias:
                    nc.tensor.matmul(
                        c_ps[:],
                        b_bf[0:1, cs * P:(cs + 1) * P],
                        rred_bf[:],
                        start=False,
                        stop=True,
                    )
                c_sb = io.tile([P, DH], f32, tag="csb")
                nc.vector.tensor_copy(c_sb[:], c_ps[:])
                nc.sync.dma_start(out_ext[cs * P:(cs + 1) * P, :], c_sb[:])

    nc.compile()
    _GRAPH_CACHE[key] = nc
    return nc


def _in_maps(hidden_states, positional_encodings, W, b):
    w_full = _decay_weights()
    wdec = np.ascontiguousarray(w_full.reshape(NT, P).T, dtype=np.float32)
    W_c = np.ascontiguousarray(W, dtype=np.float32)
    b_c = np.ascontiguousarray(b, dtype=np.float32)
    maps = []
    for c in range(NCORES):
        bi, dh = c // 2, c % 2
        maps.append(
            {
                "h": np.ascontiguousarray(hidden_states[bi], dtype=np.float32),
                "pe": np.ascontiguousarray(
                    positional_encodings[bi, :, dh * DH:(dh + 1) * DH],
                    dtype=np.float32,
                ),
                "W": W_c,
                "b": b_c,
                "wdec": wdec,
            }
        )
    return maps


def _assemble(results):
    out = np.empty((B, E, D), dtype=np.float32)
    for c in range(NCORES):
        bi, dh = c // 2, c % 2
        out[bi, :, dh * DH:(dh + 1) * DH] = results[c]["out"]
    return out


def run(hidden_states, positional_encodings, W, b, trace=False, **trace_kwargs):
    from concourse.bass_utils import run_bass_kernel_spmd

    nc = _build(bias=bool(np.any(np.asarray(b) != 0)))
    maps = _in_maps(hidden_states, positional_encodings, W, b)
    res = run_bass_kernel_spmd(
        nc, maps, core_ids=list(range(NCORES)), trace=trace, **trace_kwargs
    )
    return _assemble(res.results), res


def kernel(hidden_states, positional_encodings, W, b):
    out, _ = run(hidden_states, positional_encodings, W, b, trace=False)
    return out


# revision 63
# speedup vs baseline: 1.1147x; 1.0562x over previous
"""Trainium2 Bass kernel for nn_AttractorState (decay-weighted outer-product state).

Reference computation (per batch b):
    C[b] = sum_t alpha^(S-1-t) * (W @ h_t + bias) outer e_t        (S = 8192)

Refactored to avoid materializing the projection and to keep the big
contraction over t in natural [t, d] layout:
    G[b]  = (w . H[b])^T @ PE[b]          # [d_model=512, d_model=512], w_t = alpha^(S-1-t)
    r[b]  = w^T @ PE[b]                   # [512]
    C[b]  = W @ G[b] + bias outer r[b]    # [d_state=512, d_model=512]

Sharding over 8 NeuronCores: (batch=4) x (d-half=2), fully collective-free.
Each core processes ALL 8192 tokens of one batch but only its 256 d-columns
of PE: G_half = (w . H)^T @ PE[:, dhalf] accumulates locally in PSUM, then
C_half = W @ G_half (+ b outer r_half), writing a (512, 256) slab.  The host
reassembles (4, 512, 512).  The t-contraction never crosses cores, so no
reduction, no collectives, no inter-core sync.

The bias path (b != 0) needs an extra rank-1 accumulation r = w^T @ PE and a
per-token M=1 matmul; setup_inputs() always produces b == 0, so the default
graph skips it entirely and a bias-capable graph is built only if a nonzero
b ever shows up.

Matmul operands are cast to bf16 on-chip (DVE/ACT, overlapped with DMA);
accumulation is fp32 in PSUM.
"""

import math
import sys

import numpy as np

for _p in ("/opt/trn_rl_repo", "/opt/trn_rl_repo/concourse"):
    if _p not in sys.path:
        sys.path.append(_p)

# Problem constants (hardcoded per harness contract).
B = 4
S = 8192
D = 512          # d_model
E = 512          # d_state
P = 128          # SBUF partitions
NCORES = 8
DH = D // 2      # 256, d-half owned per core
NT = S // P      # 64 t-tiles per core
# Chunk sizes (in 128-token t-tiles): small leading chunks so matmuls start
# early, tapering tail so the PE lag after the last DMA is tiny.  The last
# RES t-tiles are a "reserve" chunk: DMA'd + scaled early, matmuls deferred
# to the very end — a dense dependency-free burst that decouples the PE tail
# from the last DMA arrival.
RES = 6
SIZES = [1, 2, 3, 4, 6, 6, 6, 6, 6, 6, 6, 4, 2]
assert sum(SIZES) == NT - RES
CH = len(SIZES)
TPC = max(SIZES)  # buffer sizing

_GRAPH_CACHE = {}


def _decay_weights():
    # Match reference: alpha = f32(exp(-pi/S)); w = exp((S-1-t) * log(alpha)) in f32.
    alpha = np.float32(math.exp(-math.pi / S))
    t = np.arange(S, dtype=np.float32)
    w = np.exp((np.float32(S - 1.0) - t) * np.log(alpha)).astype(np.float32)
    return w


def _build(bias: bool):
    key = ("bias" if bias else "nobias")
    if key in _GRAPH_CACHE:
        return _GRAPH_CACHE[key]

    import concourse.bass as bass  # noqa: F401
    import concourse.mybir as mybir
    import concourse.tile as tile
    from concourse import bacc

    f32 = mybir.dt.float32
    bf16 = mybir.dt.bfloat16
    AF = mybir.ActivationFunctionType

    nc = bacc.Bacc("TRN2", target_bir_lowering=False)

    h_ext = nc.declare_dram_parameter("h", [S, D], f32, isOutput=False)
    pe_ext = nc.declare_dram_parameter("pe", [S, DH], f32, isOutput=False)
    w_ext = nc.declare_dram_parameter("W", [E, D], f32, isOutput=False)
    b_ext = nc.declare_dram_parameter("b", [E], f32, isOutput=False)
    wdec_ext = nc.declare_dram_parameter("wdec", [P, NT], f32, isOutput=False)
    out_ext = nc.declare_dram_parameter("out", [E, DH], f32, isOutput=True)

    import ml_dtypes

    ident = nc.inline_tensor(np.eye(P, dtype=ml_dtypes.bfloat16), "ident")

    h_re = h_ext.ap().rearrange("(n p) d -> p n d", p=P)
    pe_re = pe_ext.ap().rearrange("(n p) d -> p n d", p=P)
    w_re = w_ext.ap().rearrange("(c p) e -> p c e", p=P)

    with tile.TileContext(nc) as tc:
        with (
            tc.tile_pool(name="consts", bufs=1) as consts,
            tc.tile_pool(name="io", bufs=4) as io,
            tc.tile_pool(name="acc", bufs=1, space="PSUM") as acc_pool,
            tc.tile_pool(name="pst", bufs=2, space="PSUM") as pst,
        ):
            # ---- first input chunks go out before anything else ----
            # (wdec rides the otherwise-idle gpsimd SWDGE so h0 leads the sync ring)
            wdec_sb = consts.tile([P, NT], f32)
            nc.gpsimd.dma_start(wdec_sb[:], wdec_ext[:, :])
            starts = [sum(SIZES[:j]) for j in range(CH)]
            h_tiles = []
            pe_tiles = []
            PRE = 4
            for j in range(PRE):
                n0, w = starts[j], SIZES[j]
                h_t = io.tile([P, TPC, D], f32, tag="h", name=f"h_t{j}")
                pe_t = io.tile([P, TPC, DH], f32, tag="pe", name=f"pe_t{j}")
                nc.sync.dma_start(h_t[:, 0:w, :], h_re[:, n0:n0 + w, :])
                nc.scalar.dma_start(pe_t[:, 0:w, :], pe_re[:, n0:n0 + w, :])
                h_tiles.append(h_t)
                pe_tiles.append(pe_t)

            # ---- reserve chunk: tiles [NT-RES, NT), fetched + prepped early ----
            R0 = NT - RES
            h_r = io.tile([P, RES, D], f32, tag="hr", bufs=1)
            pe_r = io.tile([P, RES, DH], f32, tag="per", bufs=1)
            nc.gpsimd.dma_start(h_r[:], h_re[:, R0:NT, :])
            nc.gpsimd.dma_start(pe_r[:], pe_re[:, R0:NT, :])
            hw_r = io.tile([P, RES, D], bf16, tag="hwr", bufs=1)
            pew_r = io.tile([P, RES, DH], bf16, tag="pewr", bufs=1)
            nc.vector.tensor_tensor(
                out=hw_r[:],
                in0=h_r[:],
                in1=wdec_sb[:, R0:NT].unsqueeze(-1).to_broadcast((P, RES, D)),
                op=mybir.AluOpType.mult,
            )
            nc.scalar.activation(pew_r[:], pe_r[:], AF.Copy)

            # ---- constants ----
            ident_sb = consts.tile([P, P], bf16)
            nc.sync.dma_start(ident_sb[:], ident[:, :])
            w_sb = consts.tile([P, 4, D], f32)      # w_sb[p, c, e] = W[c*128+p, e]
            nc.sync.dma_start(w_sb[:], w_re)
            w_bf = consts.tile([P, 4, D], bf16)
            nc.vector.tensor_copy(w_bf[:], w_sb[:])
            if bias:
                wdec_bf = consts.tile([P, NT], bf16)
                nc.vector.tensor_copy(wdec_bf[:], wdec_sb[:])
                b_sb = consts.tile([1, E], f32)
                nc.sync.dma_start(b_sb[:], b_ext.ap().unsqueeze(0))
                b_bf = consts.tile([1, E], bf16)
                nc.vector.tensor_copy(b_bf[:], b_sb[:])

            # ---- G (/ r) accumulation over all 8192 tokens ----
            g_ps = [
                acc_pool.tile([P, DH], f32, tag=f"g{k}", name=f"g_ps{k}")
                for k in range(4)
            ]
            if bias:
                r_ps = acc_pool.tile([1, DH], f32, tag="r")

            for j in range(CH):
                n0, w = starts[j], SIZES[j]
                if j < PRE:
                    h_t, pe_t = h_tiles[j], pe_tiles[j]
                else:
                    h_t = io.tile([P, TPC, D], f32, tag="h")
                    pe_t = io.tile([P, TPC, DH], f32, tag="pe")
                    nc.sync.dma_start(h_t[:, 0:w, :], h_re[:, n0:n0 + w, :])
                    nc.scalar.dma_start(pe_t[:, 0:w, :], pe_re[:, n0:n0 + w, :])
                hw_t = io.tile([P, TPC, D], bf16, tag="hw")
                pew_t = io.tile([P, TPC, DH], bf16, tag="pew")
                # whole-chunk decay-scale of h on DVE (wdec broadcast along d)
                # and whole-chunk cast of pe on ACT
                nc.vector.tensor_tensor(
                    out=hw_t[:, 0:w, :],
                    in0=h_t[:, 0:w, :],
                    in1=wdec_sb[:, n0:n0 + w].unsqueeze(-1).to_broadcast((P, w, D)),
                    op=mybir.AluOpType.mult,
                )
                nc.scalar.activation(pew_t[:, 0:w, :], pe_t[:, 0:w, :], AF.Copy)
                for i in range(w):
                    n = n0 + i
                    first = n == 0
                    for k in range(4):
                        nc.tensor.matmul(
                            g_ps[k][:],
                            hw_t[:, i, k * P:(k + 1) * P],
                            pew_t[:, i, :],
                            start=first,
                            stop=False,
                        )
                    if bias:
                        nc.tensor.matmul(
                            r_ps[:],
                            wdec_bf[:, n:n + 1],
                            pew_t[:, i, :],
                            start=first,
                            stop=False,
                        )
                if j == 0:
                    # W^T transposes (bf16), scheduled after the first chunk's matmuls
                    wt_sb = consts.tile([P, 4, E], bf16)  # wt_sb[p,c,s] = W[s, c*128+p]
                    for ce in range(4):
                        for cs in range(4):
                            tp = pst.tile([P, P], bf16, tag="tp")
                            nc.tensor.transpose(
                                tp[:], w_bf[:, cs, ce * P:(ce + 1) * P], ident_sb[:]
                            )
                            nc.vector.tensor_copy(
                                wt_sb[:, ce, cs * P:(cs + 1) * P], tp[:]
                            )

            # ---- reserve chunk matmuls: dense, all inputs long resident ----
            for i in range(RES):
                n = R0 + i
                last = n == NT - 1
                for k in range(4):
                    nc.tensor.matmul(
                        g_ps[k][:],
                        hw_r[:, i, k * P:(k + 1) * P],
                        pew_r[:, i, :],
                        start=False,
                        stop=last,
                    )
                if bias:
                    nc.tensor.matmul(
                        r_ps[:],
                        wdec_bf[:, n:n + 1],
                        pew_r[:, i, :],
                        start=False,
                        stop=last,
                    )

            # ---- G -> SBUF (bf16) ----
            gred_bf = consts.tile([P, 4, DH], bf16)
            for k in range(4):
                nc.vector.tensor_copy(gred_bf[:, k, :], g_ps[k][:])
            if bias:
                rred_bf = consts.tile([1, DH], bf16)
                nc.vector.tensor_copy(rred_bf[:], r_ps[:])

            # ---- finish C_half = W @ G_half (+ b outer r_half) ----
            for cs in range(4):
                c_ps = pst.tile([P, DH], f32, tag="cps", bufs=1)
                for ce in range(4):
                    nc.tensor.matmul(
                        c_ps[:],
                        wt_sb[:, ce, cs * P:(cs + 1) * P],
                        gred_bf[:, ce, :],
                        start=(ce == 0),
                        stop=(not bias and ce == 3),
                    )
                if bias:
                    nc.tensor.matmul(
                        c_ps[:],
                        b_bf[0:1, cs * P:(cs + 1) * P],
                        rred_bf[:],
                        start=False,
                        stop=True,
                    )
                c_sb = io.tile([P, DH], f32, tag="csb")
                nc.vector.tensor_copy(c_sb[:], c_ps[:])
                nc.sync.dma_start(out_ext[cs * P:(cs + 1) * P, :], c_sb[:])

    nc.compile()
    _GRAPH_CACHE[key] = nc
    return nc


def _in_maps(hidden_states, positional_encodings, W, b):
    w_full = _decay_weights()
    wdec = np.ascontiguousarray(w_full.reshape(NT, P).T, dtype=np.float32)
    W_c = np.ascontiguousarray(W, dtype=np.float32)
    b_c = np.ascontiguousarray(b, dtype=np.float32)
    maps = []
    for c in range(NCORES):
        bi, dh = c // 2, c % 2
        maps.append(
            {
                "h": np.ascontiguousarray(hidden_states[bi], dtype=np.float32),
                "pe": np.ascontiguousarray(
                    positional_encodings[bi, :, dh * DH:(dh + 1) * DH],
                    dtype=np.float32,
                ),
                "W": W_c,
                "b": b_c,
                "wdec": wdec,
            }
        )
    return maps


def _assemble(results):
    out = np.empty((B, E, D), dtype=np.float32)
    for c in range(NCORES):
        bi, dh = c // 2, c % 2
        out[bi, :, dh * DH:(dh + 1) * DH] = results[c]["out"]
    return out


def run(hidden_states, positional_encodings, W, b, trace=False, **trace_kwargs):
    from concourse.bass_utils import run_bass_kernel_spmd

    nc = _build(bias=bool(np.any(np.asarray(b) != 0)))
    maps = _in_maps(hidden_states, positional_encodings, W, b)
    res = run_bass_kernel_spmd(
        nc, maps, core_ids=list(range(NCORES)), trace=trace, **trace_kwargs
    )
    return _assemble(res.results), res


def kernel(hidden_states, positional_encodings, W, b):
    out, _ = run(hidden_states, positional_encodings, W, b, trace=False)
    return out
